# revision 53
# baseline (speedup 1.0000x reference)
"""Trainium2 Bass kernel for nn_Attention_85710367359290 (sparse branch-routed attention).

Semantics (same numerics as the validated baseline, rel err ~9e-3):
  q = rope(a @ Wq) per branch (NB=4), k = rope(x @ Wk) (Wk pre-scaled 1/sqrt(C)),
  v = a @ Wv per branch (bf16)
  att[b,n,t,s] = q.k;  attmax = max_n att;  p = exp(attmax) (no max-sub, |att|<~8)
  routing: cmb_n = (att_n >= attmax) * p * causal_mask
  y = sum_n v_n^T-contracted cmb_n;  Z = sum_s p;  out = (y/Z) @ Wo

Two-launch SPMD over 8 cores (host reshuffle between launches is free):
  Kernel A (small): core (b, s4) owns contiguous s-rows [512*s4, 512*(s4+1))
    of batch b; computes kr (roped kT, f32) and v (bf16) for those rows only.
  Kernel B (big): core (b, j) owns four 128-row t-chunks {15-j, 8+j, 7-j, j}
    (causally balanced: exact trips 16-j, 9+j, 8-j, j+1; program padded to the
    uniform (16, 12, 8, 4)).  q-projection + rope run ON-CORE (kills the qr
    DRAM round-trip of the baseline), then a single DESCENDING s-loop streams
    v once and runs QK / routing / PV for every active slot.  The s-loop is
    software-pipelined in 5 stages, one pipeline block apart, so no in-order
    engine head-of-line blocks on a same-block cross-engine result:
      a: QK (PE) + att PSUM->SBUF copy (Act)   [Pool can't read PSUM]
      b: branch-pair max + max (DVE)
      c: exp (Act) + is_ge vs broadcast amax (DVE)  [Pool can't is_ge]
      d: mask-mul + cmb = mb*p_m (DVE)
      back: Z column + 16 PV matmuls (PE), 4 blocks after its QK.
    Causal masks for the 16 core-dependent (si, slot) instances ship as
    bf16 data (the msk DMA must be EMITTED before the first masked
    instance's ops: tile reads sequence against writes in emission order).

All DRAM tensors are host-packed so every DMA is one [128, N] contiguous
transfer.  PSUM banks: 4 yT + 1 Z + 3 shared work (att + q-proj psums) = 8.
"""

import numpy as np
import ml_dtypes

import concourse.mybir as mybir
import concourse.tile as tile
from concourse import bacc
from concourse.bass_utils import run_bass_kernel_spmd

F32 = mybir.dt.float32
F32R = mybir.dt.float32r
BF16 = mybir.dt.bfloat16
ALU = mybir.AluOpType
ACTF = mybir.ActivationFunctionType
AXIS = mybir.AxisListType

B, T, C, NB = 2, 2048, 512, 4
N_CORES = 8
NPVD = ml_dtypes.bfloat16

TRIPS = (16, 12, 8, 4)          # padded s-trips per slot (uniform program)
SLOT_START = (15, 11, 7, 3)     # first (descending) si per slot
MASKED = [(si, slot) for slot, tr in enumerate(TRIPS)
          for si in range(tr - 4, tr)]
MIDX = {inst: i for i, inst in enumerate(MASKED)}
PIPE_DEPTH = 2                  # instances between QK and its Z+PV

_cache = {}


def _chunks_of(j):
    """t-chunks (128 rows each) owned by core j, slot-major."""
    return [15 - j, 8 + j, 7 - j, j]


# ---------------------------------------------------------------------------
# Kernel A: kr (roped k^T, f32) + v (bf16) for a 512-row s-slice
# ---------------------------------------------------------------------------
def build_kernel_a():
    if "a" in _cache:
        return _cache["a"]
    nc = bacc.Bacc("TRN2", target_bir_lowering=False, debug=False)

    def din(name, shape, dt):
        return nc.dram_tensor(name, shape, dt, kind="ExternalInput").ap()

    xTp = din("xTp", [128, 4 * 512], F32R)       # x[b].T slice, Kc-packed
    Wkp = din("Wkp", [128, 4 * 512], F32R)       # split-permuted, pre-scaled
    aTbp = din("aTbp", [128, 4 * 512], BF16)     # a[b].T slice bf16, Kc-packed
    Wvp = din("Wvp", [128, 4 * NB * 512], BF16)  # Kc-packed
    cosA = din("cosA", [128, 2 * 512], F32)      # (h, own-s) packed
    sinA = din("sinA", [128, 2 * 512], F32)
    krO = nc.dram_tensor("krO", [128, 4 * 512], F32, kind="ExternalOutput").ap()
    vO = nc.dram_tensor("vO", [128, 4 * NB * 512], BF16, kind="ExternalOutput").ap()

    with tile.TileContext(nc) as tc:
        with (
            tc.tile_pool(name="pa", bufs=1) as pa,
            tc.tile_pool(name="pat", bufs=6) as pat,
            tc.tile_pool(name="kpsP", bufs=2, space="PSUM") as pk,
            tc.tile_pool(name="vpsP", bufs=4, space="PSUM") as pvp,
        ):
            xT = pa.tile([128, 4 * 512], F32R, tag="xT", name="xT")
            Wk = pa.tile([128, 4 * 512], F32R, tag="Wk", name="Wk")
            aTb = pa.tile([128, 4 * 512], BF16, tag="aTb", name="aTb")
            Wv = pa.tile([128, 4 * NB * 512], BF16, tag="Wv", name="Wv")
            cs = pa.tile([128, 2 * 512], F32, tag="cs", name="cs")
            sn = pa.tile([128, 2 * 512], F32, tag="sn", name="sn")
            krS = pa.tile([128, 4 * 512], F32, tag="krS", name="krS")
            vS = pa.tile([128, 4 * NB * 512], BF16, tag="vS", name="vS")

            # strict single-queue priority: k-proj inputs, rope tables,
            # then v-proj inputs
            for Kc in range(4):
                nc.sync.dma_start(out=xT[:, Kc * 512:(Kc + 1) * 512],
                                  in_=xTp[:, Kc * 512:(Kc + 1) * 512])
                nc.sync.dma_start(out=Wk[:, Kc * 512:(Kc + 1) * 512],
                                  in_=Wkp[:, Kc * 512:(Kc + 1) * 512])
            for Kc in range(4):
                nc.sync.dma_start(out=aTb[:, Kc * 512:(Kc + 1) * 512],
                                  in_=aTbp[:, Kc * 512:(Kc + 1) * 512])
            for nb in range(NB):
                nc.sync.dma_start(
                    out=Wv[:, nb * 4 * 512:(nb + 1) * 4 * 512],
                    in_=Wvp[:, nb * 4 * 512:(nb + 1) * 4 * 512])
                if nb == 1:
                    nc.sync.dma_start(out=cs, in_=cosA)
                    nc.sync.dma_start(out=sn, in_=sinA)

            # ---- k projection: kpre[m] = [c'-chunk m, s] ----
            kpre = [pa.tile([128, 512], F32, tag=f"kpre{m}", name=f"kpre{m}")
                    for m in range(4)]
            for m in range(4):
                ps = pk.tile([128, 512], F32, tag="kps", name="kps")
                for Kc in range(4):
                    nc.tensor.matmul(
                        ps, Wk[:, Kc * 512 + m * 128:Kc * 512 + (m + 1) * 128],
                        xT[:, Kc * 512:(Kc + 1) * 512],
                        start=(Kc == 0), stop=(Kc == 3))
                nc.scalar.copy(out=kpre[m], in_=ps)
            # ---- k rope (split layout: row-chunk h pairs with 2+h) ----
            for h in range(2):
                csh = cs[:, h * 512:(h + 1) * 512]
                snh = sn[:, h * 512:(h + 1) * 512]
                t1 = pat.tile([128, 512], F32, tag="t1", name="t1")
                t2 = pat.tile([128, 512], F32, tag="t2", name="t2")
                nc.vector.tensor_mul(t1, kpre[h], csh)
                nc.vector.tensor_mul(t2, kpre[2 + h], snh)
                nc.vector.tensor_sub(krS[:, h * 512:(h + 1) * 512], t1, t2)
                t3 = pat.tile([128, 512], F32, tag="t3", name="t3")
                t4 = pat.tile([128, 512], F32, tag="t4", name="t4")
                nc.gpsimd.tensor_mul(t3, kpre[h], snh)
                nc.gpsimd.tensor_mul(t4, kpre[2 + h], csh)
                nc.vector.tensor_add(krS[:, (2 + h) * 512:(3 + h) * 512], t3, t4)
            nc.sync.dma_start(out=krO, in_=krS)

            # ---- v projection: nb-outer so each branch streams as its
            # Wv block (branch-major layout) arrives ----
            for nb in range(NB):
                for sc in range(4):
                    ps = pvp.tile([128, 512], F32, tag="vps", name="vps")
                    for Kc in range(4):
                        nc.tensor.matmul(
                            ps, aTb[:, Kc * 512 + sc * 128:Kc * 512 + (sc + 1) * 128],
                            Wv[:, nb * 4 * 512 + Kc * 512:nb * 4 * 512 + (Kc + 1) * 512],
                            start=(Kc == 0), stop=(Kc == 3))
                    dst = vS[:, sc * NB * 512 + nb * 512:sc * NB * 512 + (nb + 1) * 512]
                    if sc % 2 == 0:
                        nc.scalar.copy(out=dst, in_=ps)
                    else:
                        nc.vector.tensor_copy(dst, ps)
                    if nb % 2 == 1:
                        # stream out this s-sub's finished branch-pair
                        base = sc * NB * 512 + (nb - 1) * 512
                        nc.sync.dma_start(out=vO[:, base:base + 1024],
                                          in_=vS[:, base:base + 1024])
    nc.compile()
    _cache["a"] = nc
    return nc


# ---------------------------------------------------------------------------
# Kernel B: on-core q-proj + rope, then pipelined descending s-loop attention
# ---------------------------------------------------------------------------
def build_kernel_b():
    if "b" in _cache:
        return _cache["b"]
    nc = bacc.Bacc("TRN2", target_bir_lowering=False, debug=False)

    def din(name, shape, dt):
        return nc.dram_tensor(name, shape, dt, kind="ExternalInput").ap()

    aTq = din("aTq", [128, 4 * 512], F32R)       # a[b].T own-t cols, Kc-packed
    Wqp = din("Wqp", [128, NB * 4 * 512], F32R)  # (br, Kc, c') packed
    cosB = din("cosB", [128, 2 * 512], F32)      # (h, own-t)
    sinB = din("sinB", [128, 2 * 512], F32)
    krB = din("krB", [128, 16 * 512], F32R)      # kr, (si, r, s) packed
    vB = din("vB", [128, 16 * NB * 512], BF16)   # v full batch, s-chunk packed
    mskD = din("mskD", [128, len(MASKED) * 128], BF16)
    Wop = din("Wop", [128, 4 * 512], BF16)       # Kc-packed
    oO = nc.dram_tensor("oO", [4 * 128, C], F32, kind="ExternalOutput").ap()

    with tile.TileContext(nc) as tc:
        with (
            tc.tile_pool(name="pers", bufs=1) as pp,
            tc.tile_pool(name="vstr", bufs=6) as pv,
            tc.tile_pool(name="qpre_p", bufs=4) as pq,
            tc.tile_pool(name="rt", bufs=3) as prt,
            tc.tile_pool(name="att_s", bufs=1) as pas,
            tc.tile_pool(name="accP", bufs=1, space="PSUM") as pacc,
        ):
            # two tiles per r-chunk: slot-pair {0,1} (h2=0) and {2,3}
            # (h2=1) — keeps QK reads independent of deferred rope writes
            qT = [[pp.tile([128, 2 * 512], F32R, tag=f"qT{r}_{h2}",
                           name=f"qT{r}_{h2}") for h2 in range(2)]
                  for r in range(4)]
            krT = pp.tile([128, 16 * 512], F32R, tag="krT", name="krT")
            aT = pp.tile([128, 4 * 512], F32R, tag="aT", name="aT")
            cs = pp.tile([128, 2 * 512], F32, tag="cs", name="cs")
            sn = pp.tile([128, 2 * 512], F32, tag="sn", name="sn")
            msk = pp.tile([128, len(MASKED) * 128], BF16, tag="msk", name="msk")
            Wo = pp.tile([128, 4 * 512], BF16, tag="Wo", name="Wo")
            ones = pp.tile([128, 1], BF16, tag="ones", name="ones")
            warm = pp.tile([128, 1], F32, tag="warm", name="warm")
            nc.vector.memset(ones, 1.0)
            nc.vector.memset(warm, 0.0)
            nc.scalar.activation(out=warm, in_=warm, func=ACTF.Exp)  # table warmup

            yT = [pacc.tile([128, 512], F32, tag=f"yT{s}", name=f"yT{s}")
                  for s in range(4)]
            Zp = pacc.tile([128, 8], F32, tag="Zp", name="Zp")

            # DMA order (single queue = strict priority): aT, Wq per
            # branch (q-proj of branch br starts once its 1MB arrives),
            # rope tables, masks; kr/v stream per-si inside the loop.
            for Kc in range(4):
                nc.sync.dma_start(out=aT[:, Kc * 512:(Kc + 1) * 512],
                                  in_=aTq[:, Kc * 512:(Kc + 1) * 512])
            Wqpv = Wqp.rearrange("p (br k m c) -> p br k m c", br=NB, k=4, m=2)

            qTv = [[qT[r][h2].rearrange("p (slot br t) -> p slot br t",
                                        slot=2, br=NB) for h2 in range(2)]
                   for r in range(4)]
            rope_rr = [nc.vector, nc.gpsimd, nc.vector,
                       nc.vector, nc.gpsimd, nc.vector]

            def emit_qpsum(h2, br):
                """Projection psums for branch br, slot-pair {2h2, 2h2+1}.

                One [128,512] PSUM bank per m-pair: zeroed once (start on the
                first matmul), both halves accumulate, one Act copy drains."""
                qpre = pq.tile([128, 1024], F32, tag="qpre", name=f"qpre{h2}{br}")
                for mp in range(2):
                    ps = pwk.tile([128, 512], F32, tag="wk", name="qps")
                    for half in range(2):
                        m = 2 * mp + half
                        for Kc in range(4):
                            nc.tensor.matmul(
                                ps[:, half * 256:(half + 1) * 256],
                                Wq[:, br * 2048 + Kc * 512 + m * 128:
                                   br * 2048 + Kc * 512 + (m + 1) * 128],
                                aT[:, Kc * 512 + h2 * 256:Kc * 512 + h2 * 256 + 256],
                                start=(half == 0 and Kc == 0),
                                stop=(half == 1 and Kc == 3))
                    nc.scalar.copy(out=qpre[:, mp * 512:(mp + 1) * 512], in_=ps)
                return qpre

            def emit_qrope(h2, br, qpre, engs=None):
                if engs is None:
                    engs = rope_rr
                qpv = qpre.rearrange("p (m t) -> p m t", m=4)
                for h in range(2):
                    csh = cs[:, h * 512 + h2 * 256:h * 512 + h2 * 256 + 256]
                    snh = sn[:, h * 512 + h2 * 256:h * 512 + h2 * 256 + 256]
                    x1 = qpv[:, h, :]
                    x2 = qpv[:, 2 + h, :]
                    o1 = qTv[h][h2][:, :, br, :]
                    o2 = qTv[2 + h][h2][:, :, br, :]
                    t1 = prt.tile([128, 256], F32, tag="t1", name="t1")
                    t2 = prt.tile([128, 256], F32, tag="t2", name="t2")
                    engs[0].tensor_mul(t1, x1, csh)
                    engs[1].tensor_mul(t2, x2, snh)
                    engs[2].tensor_sub(
                        o1, t1.rearrange("p (a t) -> p a t", a=2),
                        t2.rearrange("p (a t) -> p a t", a=2))
                    t3 = prt.tile([128, 256], F32, tag="t3", name="t3")
                    t4 = prt.tile([128, 256], F32, tag="t4", name="t4")
                    engs[3].tensor_mul(t3, x1, snh)
                    engs[4].tensor_mul(t4, x2, csh)
                    engs[5].tensor_add(
                        o2, t3.rearrange("p (a t) -> p a t", a=2),
                        t4.rearrange("p (a t) -> p a t", a=2))

            def emit_stage_a(si, slot, idx):
                """QK (PE) + att copy to SBUF (Act).  Pool cannot read PSUM
                and cannot run is_ge, so routing works on the SBUF copy:
                Pool max-tree -> Act exp -> DVE is_ge/cmb, one pipeline block
                per hop so no engine head-of-line blocks on a same-block
                cross-engine result."""
                att = pwk.tile([128, 512], F32, tag="wk", name=f"att_{si}_{slot}")
                for Kc in range(4):
                    nc.tensor.matmul(
                        att, krT[:, si * 512 + Kc * 128:si * 512 + (Kc + 1) * 128],
                        qT[Kc][slot // 2][:, (slot % 2) * 512:(slot % 2) * 512 + 512],
                        start=(Kc == 0), stop=(Kc == 3))
                attS = pat2.tile([128, 512], F32, tag="attS", name="attS", bufs=4)
                nc.scalar.copy(out=attS, in_=att)
                return dict(si=si, slot=slot, attS=attS)

            def emit_stage_b(st):
                attS = st["attS"]
                pmax2 = pas.tile([128, 256], F32, tag="pmax2", name="pmax2",
                                 bufs=3)
                nc.vector.tensor_max(pmax2, attS[:, 0:256], attS[:, 256:512])
                amax = pas.tile([128, 128], F32, tag="amax", name="amax", bufs=3)
                nc.vector.tensor_max(amax, pmax2[:, 0:128], pmax2[:, 128:256])
                st["amax"] = amax

            def emit_stage_c(st):
                attS, amax = st["attS"], st["amax"]
                p_m = pas.tile([128, 128], BF16, tag="p_m", name="p_m", bufs=6)
                if (st["si"], st["slot"]) in MIDX:
                    e128 = pas.tile([128, 128], BF16, tag="e128", name="e128",
                                    bufs=3)
                    nc.scalar.activation(out=e128, in_=amax, func=ACTF.Exp)
                else:
                    e128 = None
                    nc.scalar.activation(out=p_m, in_=amax, func=ACTF.Exp)
                mb = pas.tile([128, 512], BF16, tag="mb", name="mb", bufs=4)
                amax_b = amax.rearrange("p (o t) -> p o t", o=1) \
                             .to_broadcast([128, NB, 128])
                nc.vector.tensor_tensor(
                    out=mb.rearrange("p (br t) -> p br t", br=NB),
                    in0=attS.rearrange("p (br t) -> p br t", br=NB),
                    in1=amax_b, op=ALU.is_ge)
                st["p_m"] = p_m
                st["e128"] = e128
                st["mb"] = mb

            def emit_stage_d(st):
                if st["e128"] is not None:
                    k = MIDX[(st["si"], st["slot"])]
                    nc.vector.tensor_mul(
                        st["p_m"], st["e128"], msk[:, k * 128:(k + 1) * 128])
                cmb = pas.tile([128, 512], BF16, tag="cmb", name="cmb", bufs=5)
                p_m_b = st["p_m"].rearrange("p (o t) -> p o t", o=1) \
                                 .to_broadcast([128, NB, 128])
                nc.vector.tensor_mul(
                    cmb.rearrange("p (br t) -> p br t", br=NB),
                    st["mb"].rearrange("p (br t) -> p br t", br=NB), p_m_b)
                st["cmb"] = cmb

            def emit_back(st):
                si, slot, p_m, cmb, vt = (st["si"], st["slot"], st["p_m"],
                                          st["cmb"], st["vt"])
                first = (si == SLOT_START[slot])
                nc.tensor.matmul(
                    Zp[:, slot:slot + 1], p_m, ones,
                    start=(si == 15 and slot == 0), stop=(si == 0 and slot == 3))
                for br in range(NB):
                    for Mc in range(4):
                        nc.tensor.matmul(
                            yT[slot][:, Mc * 128:(Mc + 1) * 128],
                            vt[:, br * 512 + Mc * 128:br * 512 + (Mc + 1) * 128],
                            cmb[:, br * 128:(br + 1) * 128],
                            start=(first and br == 0 and Mc == 0),
                            stop=(si == 0 and br == NB - 1 and Mc == 3))

            # ---- emission: q-proj upfront, then 5-stage pipelined s-loop ----
            L1, L2, L3, L4 = [], [], [], []

            def advance(new_st):
                for st in L4:
                    emit_back(st)
                L4.clear()
                for st in L3:
                    emit_stage_d(st)
                    L4.append(st)
                L3.clear()
                for st in L2:
                    emit_stage_c(st)
                    L3.append(st)
                L2.clear()
                for st in L1:
                    emit_stage_b(st)
                    L2.append(st)
                L1.clear()
                if new_st is not None:
                    L1.append(new_st)

            with tc.tile_pool(name="workP", bufs=3, space="PSUM") as pwk:
                with tc.tile_pool(name="wqP", bufs=1) as pwq:
                    Wq = pwq.tile([128, NB * 4 * 512], F32R, tag="Wq", name="Wq")
                    Wqv = Wq.rearrange("p (br k m c) -> p br k m c",
                                       br=NB, k=4, m=2)
                    csv = cs.rearrange("p (h q c) -> p h q c", h=2, q=2)
                    snv = sn.rearrange("p (h q c) -> p h q c", h=2, q=2)
                    cBv = cosB.rearrange("p (h q c) -> p h q c", h=2, q=2)
                    sBv = sinB.rearrange("p (h q c) -> p h q c", h=2, q=2)
                    for br in range(NB):
                        for mp in range(2):
                            nc.sync.dma_start(out=Wqv[:, br, :, mp, :],
                                              in_=Wqpv[:, br, :, mp, :])
                        if br == 1:
                            # only the h2=0 halves of the rope tables sit in
                            # the critical DMA prefix
                            nc.sync.dma_start(out=csv[:, :, 0, :],
                                              in_=cBv[:, :, 0, :])
                            nc.sync.dma_start(out=snv[:, :, 0, :],
                                              in_=sBv[:, :, 0, :])
                    nc.sync.dma_start(out=csv[:, :, 1, :], in_=cBv[:, :, 1, :])
                    nc.sync.dma_start(out=snv[:, :, 1, :], in_=sBv[:, :, 1, :])
                    qpres = {}
                    # h2=0 psums with their ropes interleaved (rope of br
                    # starts while br+1's psums stream in), h2=1 psums after
                    # (their ropes defer into the early s-iterations).
                    for br in range(NB):
                        qpres[(0, br)] = emit_qpsum(0, br)
                        emit_qrope(0, br, qpres[(0, br)])
                    for br in range(NB):
                        qpres[(1, br)] = emit_qpsum(1, br)
                pat2_cm = tc.tile_pool(name="attS_p", bufs=1)
                pat2 = pat2_cm.__enter__()
                light = [nc.vector, nc.gpsimd, nc.gpsimd,
                         nc.vector, nc.gpsimd, nc.gpsimd]
                pend_rope = [((1, 0), light), ((1, 1), light),
                             ((1, 2), [nc.gpsimd] * 6),
                             ((1, 3), [nc.gpsimd] * 6)]
                idx = 0
                for si in range(15, -1, -1):
                    nc.sync.dma_start(
                        out=krT[:, si * 512:(si + 1) * 512],
                        in_=krB[:, si * 512:(si + 1) * 512])
                    vt = pv.tile([128, NB * 512], BF16, tag="v", name="v")
                    nc.sync.dma_start(
                        out=vt, in_=vB[:, si * NB * 512:(si + 1) * NB * 512])
                    if si == 15:
                        # must precede the first masked instance's mask-mul
                        # in EMISSION order (reads sequence against writes)
                        nc.sync.dma_start(out=msk, in_=mskD)
                    for slot in range(4):
                        if si < TRIPS[slot]:
                            st = emit_stage_a(si, slot, idx)
                            st["vt"] = vt
                            if idx < 4:
                                # head instances are DMA-gated and PE is
                                # idle: run the whole routing chain now
                                emit_stage_b(st)
                                emit_stage_c(st)
                                emit_stage_d(st)
                                L4.append(st)
                            else:
                                advance(st)
                            idx += 1
                    if pend_rope and si <= 14:
                        (h2d, brd), engs = pend_rope.pop(0)
                        emit_qrope(h2d, brd, qpres[(h2d, brd)], engs=engs)
                for _ in range(4):
                    advance(None)
                pat2_cm.__exit__(None, None, None)
                nc.sync.dma_start(out=Wo, in_=Wop)

            # ---- epilogue: o[slot] = (yT[slot]/Z) @ Wo ----
            with tc.tile_pool(name="oP", bufs=2, space="PSUM") as pop:
                zr = pas.tile([128, 8], F32, tag="zr", name="zr")
                nc.vector.reciprocal(zr[:, 0:4], Zp[:, 0:4])
                for slot in range(4):
                    yb = pas.tile([128, 512], BF16, tag="yb", name="yb", bufs=2)
                    nc.scalar.copy(out=yb, in_=yT[slot])
                    opsm = pop.tile([128, 512], F32, tag="opsm", name="opsm")
                    for Kc in range(4):
                        nc.tensor.matmul(
                            opsm, yb[:, Kc * 128:(Kc + 1) * 128],
                            Wo[:, Kc * 512:(Kc + 1) * 512],
                            start=(Kc == 0), stop=(Kc == 3))
                    osb = pas.tile([128, 512], F32, tag="osb", name="osb", bufs=2)
                    nc.vector.tensor_scalar_mul(osb, opsm, zr[:, slot:slot + 1])
                    nc.sync.dma_start(
                        out=oO[slot * 128:(slot + 1) * 128, :], in_=osb)
    nc.compile()
    _cache["b"] = nc
    return nc


# ---------------------------------------------------------------------------
# Host driver
# ---------------------------------------------------------------------------
def _pack_kc(arr, nchunk=4):
    """[nchunk*128, N] -> [128, nchunk*N] (partition-chunk packing)."""
    r, n = arr.shape
    assert r == nchunk * 128
    return np.ascontiguousarray(
        arr.reshape(nchunk, 128, n).transpose(1, 0, 2).reshape(128, nchunk * n))


def _unpack_kc(arr, nchunk=4):
    """[128, nchunk*N] -> [nchunk*128, N]."""
    p, cn = arr.shape
    n = cn // nchunk
    return np.ascontiguousarray(
        arr.reshape(128, nchunk, n).transpose(1, 0, 2).reshape(nchunk * 128, n))


def kernel(a, x, Wq, Wk, Wv, Wo, cos, sin):
    a = np.asarray(a, np.float32)
    x = np.asarray(x, np.float32)
    Wq = np.asarray(Wq, np.float32)
    Wk = np.asarray(Wk, np.float32)
    Wv = np.asarray(Wv, np.float32)
    Wo = np.asarray(Wo, np.float32)
    cos = np.asarray(cos, np.float32)
    sin = np.asarray(sin, np.float32)

    split_idx = np.r_[0:C:2, 1:C:2]
    Wq_p = np.ascontiguousarray(Wq.reshape(C, NB, C)[:, :, split_idx].reshape(C, NB * C))
    Wk_p = np.ascontiguousarray(Wk[:, split_idx] * np.float32(1.0 / np.sqrt(C)))
    Wv_b = Wv.astype(NPVD)
    Wo_b = Wo.astype(NPVD)
    cosTf = np.ascontiguousarray(cos[:T].T)   # [C/2, T]
    sinTf = np.ascontiguousarray(sin[:T].T)

    def pack_cs(tab, colsel):
        sl = np.ascontiguousarray(tab[:, colsel])  # [256, 512]
        return _pack_kc(sl, nchunk=2)

    # ---- kernel A ----
    nca = build_kernel_a()
    in_a = []
    for core in range(N_CORES):
        b, s4 = divmod(core, 4)
        sl = np.arange(512 * s4, 512 * (s4 + 1))
        aTs = np.ascontiguousarray(a[b].T[:, sl])
        in_a.append({
            "xTp": _pack_kc(np.ascontiguousarray(x[b].T[:, sl])),
            "Wkp": _pack_kc(Wk_p),
            "aTbp": _pack_kc(aTs.astype(NPVD)),
            "Wvp": np.ascontiguousarray(
                Wv_b.reshape(4, 128, NB, 512).transpose(1, 2, 0, 3)
                .reshape(128, NB * 4 * 512)),
            "cosA": pack_cs(cosTf, sl),
            "sinA": pack_cs(sinTf, sl),
        })
    res_a = run_bass_kernel_spmd(nca, in_a, list(range(N_CORES)))

    kr_full = [np.concatenate(
        [_unpack_kc(np.asarray(res_a.results[b * 4 + s]["krO"], np.float32))
         for s in range(4)], axis=1) for b in range(B)]          # [512, 2048]
    v_full = [np.concatenate(
        [_unpack_kc(np.asarray(res_a.results[b * 4 + s]["vO"])) for s in range(4)],
        axis=0) for b in range(B)]                               # [2048, 2048] bf16

    # ---- kernel B ----
    ncb = build_kernel_b()
    in_b = []
    for core in range(N_CORES):
        b, j = divmod(core, 4)
        chunks = _chunks_of(j)
        own = np.concatenate([np.arange(128 * c, 128 * (c + 1)) for c in chunks])
        mskH = np.zeros((128, len(MASKED), 128), np.float32)
        tt = np.arange(128)[None, :]
        ss = np.arange(128)[:, None]
        for k, (si, slot) in enumerate(MASKED):
            mskH[:, k, :] = (128 * chunks[slot] + tt) >= (128 * si + ss)
        in_b.append({
            "aTq": _pack_kc(np.ascontiguousarray(a[b].T[:, own])),
            "Wqp": np.ascontiguousarray(
                Wq_p.reshape(4, 128, NB, 512).transpose(1, 2, 0, 3)
                .reshape(128, NB * 4 * 512)),
            "cosB": pack_cs(cosTf, own),
            "sinB": pack_cs(sinTf, own),
            "krB": np.ascontiguousarray(
                kr_full[b].reshape(4, 128, 16, 128).transpose(1, 2, 0, 3)
                .reshape(128, 16 * 512)),
            "vB": _pack_kc(v_full[b], nchunk=16),
            "mskD": np.ascontiguousarray(
                mskH.reshape(128, len(MASKED) * 128)).astype(NPVD),
            "Wop": _pack_kc(Wo_b),
        })
    res_b = run_bass_kernel_spmd(ncb, in_b, list(range(N_CORES)))

    outf = np.zeros((B, T, C), np.float32)
    for core in range(N_CORES):
        b, j = divmod(core, 4)
        chunks = _chunks_of(j)
        o = np.asarray(res_b.results[core]["oO"], np.float32)
        for slot in range(4):
            c = chunks[slot]
            outf[b, 128 * c:128 * (c + 1)] = o[slot * 128:(slot + 1) * 128]
    return outf
